# revision 6
# baseline (speedup 1.0000x reference)
"""Binarized linear kernel for Trainium2 (8 NeuronCores).

Problem: per-direction binary "match count" GEMM.
  input        (B=64, D=128, I=512)  bool
  weight_noise (D=128, O=512, I=512) bool
  bias_noise   (D=128, O=512)        float32
  out[b,d,o] = (#matches(input[b,d,:], weight_noise[d,:,:]) > bias_noise[d,o])

Math: with +/-1 encoding x~=2x-1, w~=2w-1:
  matches = (I + sum_i x~ w~) / 2, so
  out = (dotpm > 2*bias - I), where dotpm is a single +/-1 GEMM per direction.
Host pre-encodes +/-1 in fp8_e4m3 (exact), precomputes thr = 2*bias - I (exact
in fp32: 2*bias is exact; subtracting 512 from values in [256,1024] is exact by
Sterbenz). PSUM accumulates fp32 exactly (integers <= 512), so the comparison
is bit-identical to the reference.

Sharding: D across the 8 cores (16 directions each), fully independent.

Implementation: raw bacc (no TileContext) — hand-rolled semaphore pipeline
to avoid Tile's ~15us of prologue/epilogue barriers and per-tile semaphore
churn.  Engine roles:
  Sync:   x DMA, 8x w-chunk DMAs (2 directions each), final out DMA
  GpSimd: thr row DMA + partition_broadcast to 64 partitions, sem cleanup
  PE:     per direction: 4 accumulating fp8 matmuls (K=4x128) into a PSUM bank
  DVE:    per direction: psum > thr compare -> uint8
"""

import numpy as np

import sys

for _p in ("/opt/trn_rl_repo",):
    if _p not in sys.path:
        sys.path.insert(0, _p)

B, D, O, I = 64, 128, 512, 512
NCORES = 8
DL = D // NCORES  # directions per core (16)
KC = I // 128     # contraction chunks of 128 (4)
NB = 8            # PSUM banks used (round robin over directions)

_NC_CACHE = {}


def _build_bass():
    import concourse.mybir as mybir
    from concourse import bacc
    import concourse.bass as bass

    fp8 = mybir.dt.float8e4
    f32 = mybir.dt.float32
    u8 = mybir.dt.uint8

    nc = bacc.Bacc("TRN2")
    # DRAM layouts (host pre-arranged, all DMAs fully contiguous):
    #   xt [128, (d c b)]          : xt[k, d, c, b] = xs[b, d0+d, c*128+k]
    #   wt [8, 128, (j c o)]       : wt[p, k, j, c, o] = ws[d0+2p+j, o, c*128+k]
    #   thr [1, (d o)]             : 2*bias - I
    #   out [64, (d o)]
    xt_d = nc.dram_tensor("xt", [128, DL * KC * B], fp8, kind="ExternalInput")
    wt_d = nc.dram_tensor("wt", [DL // 2, 128, 2 * KC * O], fp8, kind="ExternalInput")
    thr_d = nc.dram_tensor("thr", [1, DL * O], f32, kind="ExternalInput")
    out_d = nc.dram_tensor("out", [B, DL * O], u8, kind="ExternalOutput")

    with (
        nc.sbuf_tensor("x_sb", [128, DL * KC * B], fp8) as x_sb,
        nc.sbuf_tensor("w_sb", [128, DL * KC * O], fp8) as w_sb,
        nc.sbuf_tensor("thr_row", [1, DL * O], f32) as thr_row,
        nc.sbuf_tensor("thr_sb", [B, DL * O], f32) as thr_sb,
        nc.sbuf_tensor("out_sb", [B, DL * O], u8) as out_sb,
        nc.psum_tensor([B, NB * O], f32) as psum,
        nc.semaphore("sem_x") as sem_x,
        nc.semaphore("sem_w") as sem_w,
        nc.semaphore("sem_thr") as sem_thr,
        nc.semaphore("sem_bc") as sem_bc,
        nc.semaphore("sem_pe") as sem_pe,
        nc.semaphore("sem_dve") as sem_dve,
        nc.semaphore("sem_out") as sem_out,
        nc.Block() as block,
    ):
        xv = x_sb[:, :].rearrange("k (d c b) -> k d c b", d=DL, c=KC)
        # w_sb free layout mirrors DRAM: (p j c o), direction d = 2p + j
        wv = w_sb[:, :].rearrange("k (p j c o) -> k p j c o", p=DL // 2, j=2, c=KC)

        @block.sync
        def _(sync):
            sync.dma_start(x_sb[:, :], xt_d[:, :]).then_inc(sem_x, 16)
            for p in range(DL // 2):
                sync.dma_start(
                    w_sb[:, p * 2 * KC * O : (p + 1) * 2 * KC * O], wt_d[p, :, :]
                ).then_inc(sem_w, 16)
            sync.wait_ge(sem_dve, DL)
            sync.dma_start(out_d[:, :], out_sb[:, :]).then_inc(sem_out, 16)
            sync.wait_ge(sem_out, 16)

        @block.gpsimd
        def _(g):
            g.dma_start(thr_row[:, :], thr_d[:, :]).then_inc(sem_thr, 16)
            g.wait_ge(sem_thr, 16)
            g.partition_broadcast(thr_sb[:, :], thr_row[:, :]).then_inc(sem_bc, 1)
            # cleanup: after everything retires, reset sems so the NEFF can
            # be re-executed
            g.wait_ge(sem_out, 16)
            all_sems = [sem_x, sem_w, sem_thr, sem_bc, sem_pe, sem_dve, sem_out]
            nums = sorted(s.num for s in all_sems)
            lo, hi = nums[0], nums[-1]
            assert nums == list(range(lo, hi + 1)), nums
            g.dma_reset(range(lo, hi + 1))
            g.sem_clear(range(lo, hi + 1))

        @block.tensor
        def _(t):
            t.wait_ge(sem_x, 16)
            for d in range(DL):
                if d % 2 == 0:
                    t.wait_ge(sem_w, 16 * (d // 2 + 1))
                if d >= NB:
                    t.wait_ge(sem_dve, d - NB + 1)
                bank = d % NB
                mm = None
                for c in range(KC):
                    mm = t.matmul(
                        psum[:, bank * O : (bank + 1) * O],
                        xv[:, d, c, :],
                        wv[:, d // 2, d % 2, c, :],
                        start=(c == 0),
                        stop=(c == KC - 1),
                    )
                mm.then_inc(sem_pe, 1)

        @block.vector
        def _(v):
            v.wait_ge(sem_bc, 1)
            for d in range(DL):
                v.wait_ge(sem_pe, d + 1)
                bank = d % NB
                v.tensor_tensor(
                    out=out_sb[:, d * O : (d + 1) * O],
                    in0=psum[:, bank * O : (bank + 1) * O],
                    in1=thr_sb[:, d * O : (d + 1) * O],
                    op=mybir.AluOpType.is_gt,
                ).then_inc(sem_dve, 1)

    nc.compile()
    return nc


def _get_nc():
    if "nc" not in _NC_CACHE:
        _NC_CACHE["nc"] = _build_bass()
    return _NC_CACHE["nc"]


def _prep_inputs(input, weight_noise, bias_noise):
    import ml_dtypes

    fp8 = ml_dtypes.float8_e4m3
    x = np.asarray(input).astype(np.int8)  # (B, D, I) in {0,1}
    w = np.asarray(weight_noise).astype(np.int8)  # (D, O, I)
    bias = np.asarray(bias_noise).astype(np.float32)  # (D, O)

    xs = (2 * x - 1).astype(fp8)  # +/-1
    ws = (2 * w - 1).astype(fp8)
    thr = (np.float32(2.0) * bias - np.float32(I)).astype(np.float32)

    in_maps = []
    for cidx in range(NCORES):
        dsl = slice(cidx * DL, (cidx + 1) * DL)
        # xt[k, d, c, b] = xs[b, d0+d, c*128+k]
        xt = xs[:, dsl, :].transpose(2, 1, 0)  # (I, DL, B)
        xt = xt.reshape(KC, 128, DL, B).transpose(1, 2, 0, 3)  # (k, d, c, b)
        xt = np.ascontiguousarray(xt).reshape(128, DL * KC * B)
        # wt[p, k, j, c, o] = ws[d0+2p+j, o, c*128+k]
        wt = ws[dsl].transpose(0, 2, 1)  # (DL, I, O)
        wt = wt.reshape(DL // 2, 2, KC, 128, O)  # (p, j, c, k, o)
        wt = wt.transpose(0, 3, 1, 2, 4)  # (p, k, j, c, o)
        wt = np.ascontiguousarray(wt).reshape(DL // 2, 128, 2 * KC * O)
        th = np.ascontiguousarray(thr[dsl].reshape(1, DL * O))
        in_maps.append({"xt": xt, "wt": wt, "thr": th})
    return in_maps


def kernel(input, weight_noise, bias_noise):
    from concourse import bass_utils

    in_maps = _prep_inputs(input, weight_noise, bias_noise)
    nc = _get_nc()
    res = bass_utils.run_bass_kernel_spmd(nc, in_maps, core_ids=list(range(NCORES)))
    outs = [np.asarray(r["out"]).reshape(B, DL, O) for r in res.results]
    full = np.concatenate(outs, axis=1)  # (B, D, O)
    return full.astype(bool)


# revision 19
# speedup vs baseline: 1.2127x; 1.2127x over previous
"""Binarized linear kernel for Trainium2 (8 NeuronCores).

Problem: per-direction binary "match count" GEMM.
  input        (B=64, D=128, I=512)  bool
  weight_noise (D=128, O=512, I=512) bool
  bias_noise   (D=128, O=512)        float32
  out[b,d,o] = (#matches(input[b,d,:], weight_noise[d,:,:]) > bias_noise[d,o])

Math: with +/-1 encoding x~=2x-1, w~=2w-1:
  matches = (I + sum_i x~ w~) / 2, so
  out = (dotpm > 2*bias - I), where dotpm is a single +/-1 GEMM per direction.
Host pre-encodes +/-1 in fp8_e4m3 (exact), precomputes thr = 2*bias - I (exact
in fp32: 2*bias is exact; subtracting 512 from values in [256,1024] is exact by
Sterbenz). PSUM accumulates fp32 exactly (integers <= 512), so the comparison
is bit-identical to the reference.

Sharding: D across the 8 cores (16 directions each), fully independent.

Implementation: raw bacc (no TileContext) — hand-rolled semaphore pipeline
to avoid Tile's ~15us of prologue/epilogue barriers and per-tile semaphore
churn.  Engine roles:
  Sync:   x DMA, 8x w-chunk DMAs (2 directions each), final out DMA
  GpSimd: thr row DMA + partition_broadcast to 64 partitions, sem cleanup
  PE:     per direction: 4 accumulating fp8 matmuls (K=4x128) into a PSUM bank
  DVE:    per direction: psum > thr compare -> uint8
"""

import numpy as np

import sys

for _p in ("/opt/trn_rl_repo",):
    if _p not in sys.path:
        sys.path.insert(0, _p)

B, D, O, I = 64, 128, 512, 512
NCORES = 8
DL = D // NCORES  # directions per core (16)
KC = I // 128     # contraction chunks of 128 (4)
NB = 8            # PSUM banks used (round robin over directions)

_NC_CACHE = {}


def _build_bass():
    import concourse.mybir as mybir
    from concourse import bacc
    import concourse.bass as bass

    fp8 = mybir.dt.float8e4
    f16 = mybir.dt.float16
    f32 = mybir.dt.float32
    u8 = mybir.dt.uint8

    nc = bacc.Bacc("TRN2")
    # DRAM layouts (host pre-arranged, all DMAs fully contiguous):
    #   xt [128, (d c b)]          : xt[k, d, c, b] = xs[b, d0+d, c*128+k]
    #   wt [8, 128, (j c o)]       : wt[p, k, j, c, o] = ws[d0+2p+j, o, c*128+k]
    #   thr [1, (d o)]             : 2*bias - I
    #   out [64, (d o)]
    xt_d = nc.dram_tensor("xt", [128, DL * KC * B], fp8, kind="ExternalInput")
    wt_d = nc.dram_tensor("wt", [DL // 2, 128, 2 * KC * O], fp8, kind="ExternalInput")
    thr_d = nc.dram_tensor("thr", [B, DL * O], f16, kind="ExternalInput")
    out_d = nc.dram_tensor("out", [B, DL * O], u8, kind="ExternalOutput")

    with (
        nc.sbuf_tensor("x_sb", [128, DL * KC * B], fp8) as x_sb,
        nc.sbuf_tensor("w_sb", [128, DL * KC * O], fp8) as w_sb,
        nc.sbuf_tensor("thr_sb", [B, DL * O], f16) as thr_sb,
        nc.sbuf_tensor("out_sb", [B, DL * O], u8) as out_sb,
        nc.psum_tensor([B, NB * O], f32) as psum,
        nc.semaphore("sem_x") as sem_x,
        nc.semaphore("sem_w") as sem_w,
        nc.semaphore("sem_thr") as sem_thr,
        nc.semaphore("sem_pe") as sem_pe,
        nc.semaphore("sem_dve") as sem_dve,
        nc.semaphore("sem_out") as sem_out,
        nc.Block() as block,
    ):
        xv = x_sb[:, :].rearrange("k (d c b) -> k d c b", d=DL, c=KC)
        # w_sb free layout mirrors DRAM: (p j c o), direction d = 2p + j
        wv = w_sb[:, :].rearrange("k (p j c o) -> k p j c o", p=DL // 2, j=2, c=KC)

        @block.sync
        def _(sync):
            sync.dma_start(x_sb[:, :], xt_d[:, :]).then_inc(sem_x, 16)
            for p in range(DL // 2):
                sync.dma_start(
                    w_sb[:, p * 2 * KC * O : (p + 1) * 2 * KC * O], wt_d[p, :, :]
                ).then_inc(sem_w, 16)
            sync.wait_ge(sem_dve, DL)
            sync.dma_start(out_d[:, :], out_sb[:, :]).then_inc(sem_out, 16)
            sync.wait_ge(sem_out, 16)

        @block.gpsimd
        def _(g):
            # thr arrives host-replicated (fp16, 1MB) on the SWDGE queue so it
            # never contends with the weight stream on the sync HWDGE queue.
            g.dma_start(thr_sb[:, :], thr_d[:, :]).then_inc(sem_thr, 16)
            # cleanup: after everything retires, reset sems so the NEFF can
            # be re-executed
            g.wait_ge(sem_out, 16)
            all_sems = [sem_x, sem_w, sem_thr, sem_pe, sem_dve, sem_out]
            nums = sorted(s.num for s in all_sems)
            lo, hi = nums[0], nums[-1]
            assert nums == list(range(lo, hi + 1)), nums
            g.dma_reset(range(lo, hi + 1))
            g.sem_clear(range(lo, hi + 1))

        @block.tensor
        def _(t):
            t.wait_ge(sem_x, 16)
            for d in range(DL):
                if d % 2 == 0:
                    t.wait_ge(sem_w, 16 * (d // 2 + 1))
                if d >= NB:
                    t.wait_ge(sem_dve, d - NB + 1)
                bank = d % NB
                mm = None
                for c in range(KC):
                    mm = t.matmul(
                        psum[:, bank * O : (bank + 1) * O],
                        xv[:, d, c, :],
                        wv[:, d // 2, d % 2, c, :],
                        start=(c == 0),
                        stop=(c == KC - 1),
                    )
                mm.then_inc(sem_pe, 1)

        @block.vector
        def _(v):
            v.wait_ge(sem_thr, 16)
            for d in range(DL):
                v.wait_ge(sem_pe, d + 1)
                bank = d % NB
                v.tensor_tensor(
                    out=out_sb[:, d * O : (d + 1) * O],
                    in0=psum[:, bank * O : (bank + 1) * O],
                    in1=thr_sb[:, d * O : (d + 1) * O],
                    op=mybir.AluOpType.is_gt,
                ).then_inc(sem_dve, 1)

    nc.compile()
    return nc


def _get_nc():
    if "nc" not in _NC_CACHE:
        _NC_CACHE["nc"] = _build_bass()
    return _NC_CACHE["nc"]


def _prep_inputs(input, weight_noise, bias_noise):
    import ml_dtypes

    fp8 = ml_dtypes.float8_e4m3
    x = np.asarray(input).astype(np.int8)  # (B, D, I) in {0,1}
    w = np.asarray(weight_noise).astype(np.int8)  # (D, O, I)
    bias = np.asarray(bias_noise).astype(np.float32)  # (D, O)

    xs = (2 * x - 1).astype(fp8)  # +/-1
    ws = (2 * w - 1).astype(fp8)
    # dotpm is always an even integer, so comparing against the odd integer
    # 2*floor(thr/2)+1 gives identical results to comparing against thr —
    # and odd integers |t|<=513 are exact in fp16 (halves thr DMA traffic).
    thr = np.float32(2.0) * bias - np.float32(I)
    thr = (2.0 * np.floor(thr.astype(np.float64) / 2.0) + 1.0).astype(np.float16)

    in_maps = []
    for cidx in range(NCORES):
        dsl = slice(cidx * DL, (cidx + 1) * DL)
        # xt[k, d, c, b] = xs[b, d0+d, c*128+k]
        xt = xs[:, dsl, :].transpose(2, 1, 0)  # (I, DL, B)
        xt = xt.reshape(KC, 128, DL, B).transpose(1, 2, 0, 3)  # (k, d, c, b)
        xt = np.ascontiguousarray(xt).reshape(128, DL * KC * B)
        # wt[p, k, j, c, o] = ws[d0+2p+j, o, c*128+k]
        wt = ws[dsl].transpose(0, 2, 1)  # (DL, I, O)
        wt = wt.reshape(DL // 2, 2, KC, 128, O)  # (p, j, c, k, o)
        wt = wt.transpose(0, 3, 1, 2, 4)  # (p, k, j, c, o)
        wt = np.ascontiguousarray(wt).reshape(DL // 2, 128, 2 * KC * O)
        th = np.ascontiguousarray(
            np.broadcast_to(thr[dsl].reshape(1, DL * O), (B, DL * O))
        )
        in_maps.append({"xt": xt, "wt": wt, "thr": th})
    return in_maps


def kernel(input, weight_noise, bias_noise):
    from concourse import bass_utils

    in_maps = _prep_inputs(input, weight_noise, bias_noise)
    nc = _get_nc()
    res = bass_utils.run_bass_kernel_spmd(nc, in_maps, core_ids=list(range(NCORES)))
    outs = [np.asarray(r["out"]).reshape(B, DL, O) for r in res.results]
    full = np.concatenate(outs, axis=1)  # (B, D, O)
    return full.astype(bool)


# revision 22
# speedup vs baseline: 1.2148x; 1.0017x over previous
"""Binarized linear kernel for Trainium2 (8 NeuronCores).

Problem: per-direction binary "match count" GEMM.
  input        (B=64, D=128, I=512)  bool
  weight_noise (D=128, O=512, I=512) bool
  bias_noise   (D=128, O=512)        float32
  out[b,d,o] = (#matches(input[b,d,:], weight_noise[d,:,:]) > bias_noise[d,o])

Math: with +/-1 encoding x~=2x-1, w~=2w-1:
  matches = (I + sum_i x~ w~) / 2, so
  out = (dotpm > 2*bias - I), where dotpm is a single +/-1 GEMM per direction.
Host pre-encodes +/-1 in fp8_e4m3 (exact), precomputes thr = 2*bias - I (exact
in fp32: 2*bias is exact; subtracting 512 from values in [256,1024] is exact by
Sterbenz). PSUM accumulates fp32 exactly (integers <= 512), so the comparison
is bit-identical to the reference.

Sharding: D across the 8 cores (16 directions each), fully independent.

Implementation: raw bacc (no TileContext) — hand-rolled semaphore pipeline
to avoid Tile's ~15us of prologue/epilogue barriers and per-tile semaphore
churn.  Engine roles:
  Sync:   x DMA, 8x w-chunk DMAs (2 directions each), final out DMA
  GpSimd: thr row DMA + partition_broadcast to 64 partitions, sem cleanup
  PE:     per direction: 4 accumulating fp8 matmuls (K=4x128) into a PSUM bank
  DVE:    per direction: psum > thr compare -> uint8
"""

import numpy as np

import sys

for _p in ("/opt/trn_rl_repo",):
    if _p not in sys.path:
        sys.path.insert(0, _p)

B, D, O, I = 64, 128, 512, 512
NCORES = 8
DL = D // NCORES  # directions per core (16)
KC = I // 128     # contraction chunks of 128 (4)
NB = 8            # PSUM banks used (round robin over directions)

_NC_CACHE = {}


def _build_bass():
    import concourse.mybir as mybir
    from concourse import bacc
    import concourse.bass as bass

    fp8 = mybir.dt.float8e4
    f16 = mybir.dt.float16
    f32 = mybir.dt.float32
    u8 = mybir.dt.uint8

    nc = bacc.Bacc("TRN2")
    # DRAM layouts (host pre-arranged, all DMAs fully contiguous):
    #   xt [128, (d c b)]          : xt[k, d, c, b] = xs[b, d0+d, c*128+k]
    #   wt [8, 128, (j c o)]       : wt[p, k, j, c, o] = ws[d0+2p+j, o, c*128+k]
    #   thr [1, (d o)]             : 2*bias - I
    #   out [64, (d o)]
    xt_d = nc.dram_tensor("xt", [128, DL * KC * B], fp8, kind="ExternalInput")
    wt_d = nc.dram_tensor("wt", [DL // 2, 128, 2 * KC * O], fp8, kind="ExternalInput")
    thr_d = nc.dram_tensor("thr", [B, DL * O], f16, kind="ExternalInput")
    out_d = nc.dram_tensor("out", [B, DL * O], u8, kind="ExternalOutput")

    from contextlib import ExitStack

    with ExitStack() as ctx:
        x_sb = ctx.enter_context(nc.sbuf_tensor("x_sb", [128, DL * KC * B], fp8))
        w_sb = ctx.enter_context(nc.sbuf_tensor("w_sb", [128, DL * KC * O], fp8))
        thr_sb = ctx.enter_context(nc.sbuf_tensor("thr_sb", [B, DL * O], f16))
        out_sb = ctx.enter_context(nc.sbuf_tensor("out_sb", [B, DL * O], u8))
        warm_x = ctx.enter_context(nc.sbuf_tensor("warm_x", [128, B], fp8))
        warm_w = ctx.enter_context(nc.sbuf_tensor("warm_w", [128, O], fp8))
        psum = ctx.enter_context(nc.psum_tensor([B, NB * O], f32))
        sem_x = ctx.enter_context(nc.semaphore("sem_x"))
        sem_w = [
            ctx.enter_context(nc.semaphore(f"sem_w{k}")) for k in range(DL // 2)
        ]
        sem_thr = ctx.enter_context(nc.semaphore("sem_thr"))
        sem_pe = ctx.enter_context(nc.semaphore("sem_pe"))
        sem_dve = ctx.enter_context(nc.semaphore("sem_dve"))
        sem_out = ctx.enter_context(nc.semaphore("sem_out"))
        block = ctx.enter_context(nc.Block())
        xv = x_sb[:, :].rearrange("k (d c b) -> k d c b", d=DL, c=KC)
        # w_sb free layout mirrors DRAM: (p j c o), direction d = 2p + j
        wv = w_sb[:, :].rearrange("k (p j c o) -> k p j c o", p=DL // 2, j=2, c=KC)

        NCHUNK = DL // 2

        @block.sync
        def _(sync):
            # Issue order is completion order (DMA engines drain near-FIFO):
            # x, w0, w1, thr, then w2.. chained 2-deep so chunk completions
            # stay early and evenly spaced instead of all clustering late.
            sync.dma_start(x_sb[:, :], xt_d[:, :]).then_inc(sem_x, 16)
            for p in range(NCHUNK):
                if p == 2:
                    sync.dma_start(thr_sb[:, :], thr_d[:, :]).then_inc(sem_thr, 16)
                if p >= 2:
                    sync.wait_ge(sem_w[p - 2], 16)
                sync.dma_start(
                    w_sb[:, p * 2 * KC * O : (p + 1) * 2 * KC * O], wt_d[p, :, :]
                ).then_inc(sem_w[p], 16)
            sync.wait_ge(sem_dve, DL)
            sync.dma_start(out_d[:, :], out_sb[:, :]).then_inc(sem_out, 16)
            sync.wait_ge(sem_out, 16)

        @block.gpsimd
        def _(g):
            # cleanup: after everything retires, reset sems so the NEFF can
            # be re-executed
            g.wait_ge(sem_out, 16)
            all_sems = [sem_x, *sem_w, sem_thr, sem_pe, sem_dve, sem_out]
            nums = sorted(s.num for s in all_sems)
            lo, hi = nums[0], nums[-1]
            assert nums == list(range(lo, hi + 1)), nums
            g.dma_reset(range(lo, hi + 1))
            g.sem_clear(range(lo, hi + 1))

        N_WARM = 12

        @block.tensor
        def _(t):
            # Warm the PE HAM clock gate on throwaway operands while the
            # first weight chunk streams in (~3.4us of busy time flips the
            # clock from 1.2 to 2.4 GHz).
            for _ in range(N_WARM):
                t.matmul(
                    psum[:, (NB - 1) * O : NB * O],
                    warm_x[:, :],
                    warm_w[:, :],
                    start=True,
                    stop=True,
                )
            t.wait_ge(sem_x, 16)
            for d in range(DL):
                if d % 2 == 0:
                    t.wait_ge(sem_w[d // 2], 16)
                if d >= NB:
                    t.wait_ge(sem_dve, d - NB + 1)
                bank = d % NB
                mm = None
                for c in range(KC):
                    mm = t.matmul(
                        psum[:, bank * O : (bank + 1) * O],
                        xv[:, d, c, :],
                        wv[:, d // 2, d % 2, c, :],
                        start=(c == 0),
                        stop=(c == KC - 1),
                    )
                mm.then_inc(sem_pe, 1)

        @block.vector
        def _(v):
            v.wait_ge(sem_thr, 16)
            for d in range(DL):
                v.wait_ge(sem_pe, d + 1)
                bank = d % NB
                v.tensor_tensor(
                    out=out_sb[:, d * O : (d + 1) * O],
                    in0=psum[:, bank * O : (bank + 1) * O],
                    in1=thr_sb[:, d * O : (d + 1) * O],
                    op=mybir.AluOpType.is_gt,
                ).then_inc(sem_dve, 1)

    nc.compile()
    return nc


def _get_nc():
    if "nc" not in _NC_CACHE:
        _NC_CACHE["nc"] = _build_bass()
    return _NC_CACHE["nc"]


def _prep_inputs(input, weight_noise, bias_noise):
    import ml_dtypes

    fp8 = ml_dtypes.float8_e4m3
    x = np.asarray(input).astype(np.int8)  # (B, D, I) in {0,1}
    w = np.asarray(weight_noise).astype(np.int8)  # (D, O, I)
    bias = np.asarray(bias_noise).astype(np.float32)  # (D, O)

    xs = (2 * x - 1).astype(fp8)  # +/-1
    ws = (2 * w - 1).astype(fp8)
    # dotpm is always an even integer, so comparing against the odd integer
    # 2*floor(thr/2)+1 gives identical results to comparing against thr —
    # and odd integers |t|<=513 are exact in fp16 (halves thr DMA traffic).
    thr = np.float32(2.0) * bias - np.float32(I)
    thr = (2.0 * np.floor(thr.astype(np.float64) / 2.0) + 1.0).astype(np.float16)

    in_maps = []
    for cidx in range(NCORES):
        dsl = slice(cidx * DL, (cidx + 1) * DL)
        # xt[k, d, c, b] = xs[b, d0+d, c*128+k]
        xt = xs[:, dsl, :].transpose(2, 1, 0)  # (I, DL, B)
        xt = xt.reshape(KC, 128, DL, B).transpose(1, 2, 0, 3)  # (k, d, c, b)
        xt = np.ascontiguousarray(xt).reshape(128, DL * KC * B)
        # wt[p, k, j, c, o] = ws[d0+2p+j, o, c*128+k]
        wt = ws[dsl].transpose(0, 2, 1)  # (DL, I, O)
        wt = wt.reshape(DL // 2, 2, KC, 128, O)  # (p, j, c, k, o)
        wt = wt.transpose(0, 3, 1, 2, 4)  # (p, k, j, c, o)
        wt = np.ascontiguousarray(wt).reshape(DL // 2, 128, 2 * KC * O)
        th = np.ascontiguousarray(
            np.broadcast_to(thr[dsl].reshape(1, DL * O), (B, DL * O))
        )
        in_maps.append({"xt": xt, "wt": wt, "thr": th})
    return in_maps


def _patch_walrus_args():
    """Cap the semaphore space walrus allocates: its NEFF epilogue clears
    every allocatable semaphore one instruction at a time (~6us for 256)."""
    from concourse import bass_utils as bu

    if getattr(bu, "_max_sem_patched", False):
        return
    orig = bu.get_walrus_args

    def patched(*a, **k):
        return ["--max-sem-num=64", *orig(*a, **k)]

    bu.get_walrus_args = patched
    bu._max_sem_patched = True


def kernel(input, weight_noise, bias_noise):
    from concourse import bass_utils

    _patch_walrus_args()
    in_maps = _prep_inputs(input, weight_noise, bias_noise)
    nc = _get_nc()
    res = bass_utils.run_bass_kernel_spmd(nc, in_maps, core_ids=list(range(NCORES)))
    outs = [np.asarray(r["out"]).reshape(B, DL, O) for r in res.results]
    full = np.concatenate(outs, axis=1)  # (B, D, O)
    return full.astype(bool)


# revision 28
# speedup vs baseline: 1.2533x; 1.0317x over previous
"""Binarized linear kernel for Trainium2 (8 NeuronCores).

Problem: per-direction binary "match count" GEMM.
  input        (B=64, D=128, I=512)  bool
  weight_noise (D=128, O=512, I=512) bool
  bias_noise   (D=128, O=512)        float32
  out[b,d,o] = (#matches(input[b,d,:], weight_noise[d,:,:]) > bias_noise[d,o])

Math: with +/-1 encoding x~=2x-1, w~=2w-1:
  matches = (I + sum_i x~ w~) / 2, so
  out = (dotpm > 2*bias - I), where dotpm is a single +/-1 GEMM per direction.
Host pre-encodes +/-1 in fp8_e4m3 (exact), precomputes thr = 2*bias - I (exact
in fp32: 2*bias is exact; subtracting 512 from values in [256,1024] is exact by
Sterbenz). PSUM accumulates fp32 exactly (integers <= 512), so the comparison
is bit-identical to the reference.

Sharding: D across the 8 cores (16 directions each), fully independent.

Implementation: raw bacc (no TileContext) — hand-rolled semaphore pipeline
to avoid Tile's ~15us of prologue/epilogue barriers and per-tile semaphore
churn.  Engine roles:
  Sync:   x DMA, 8x w-chunk DMAs (2 directions each), final out DMA
  GpSimd: thr row DMA + partition_broadcast to 64 partitions, sem cleanup
  PE:     per direction: 4 accumulating fp8 matmuls (K=4x128) into a PSUM bank
  DVE:    per direction: psum > thr compare -> uint8
"""

import numpy as np

import sys

for _p in ("/opt/trn_rl_repo",):
    if _p not in sys.path:
        sys.path.insert(0, _p)

B, D, O, I = 64, 128, 512, 512
NCORES = 8
DL = D // NCORES  # directions per core (16)
KC = I // 128     # contraction chunks of 128 (4)
NB = 8            # PSUM banks used (round robin over directions)

_NC_CACHE = {}


def _build_bass():
    import concourse.mybir as mybir
    from concourse import bacc
    import concourse.bass as bass

    fp8 = mybir.dt.float8e4
    f16 = mybir.dt.float16
    f32 = mybir.dt.float32
    u8 = mybir.dt.uint8

    nc = bacc.Bacc("TRN2")
    # DRAM layouts (host pre-arranged, all DMAs fully contiguous):
    #   xt [128, (d c b)]          : xt[k, d, c, b] = xs[b, d0+d, c*128+k]
    #   wt [8, 128, (j c o)]       : wt[p, k, j, c, o] = ws[d0+2p+j, o, c*128+k]
    #   thr [1, (d o)]             : 2*bias - I
    #   out [64, (d o)]
    xt_d = nc.dram_tensor("xt", [128, DL * KC * B], fp8, kind="ExternalInput")
    wt_d = nc.dram_tensor("wt", [DL, 128, KC * O], fp8, kind="ExternalInput")
    thr_d = nc.dram_tensor("thr", [B, DL * O], f16, kind="ExternalInput")
    out_d = nc.dram_tensor("out", [B, DL * O], u8, kind="ExternalOutput")

    from contextlib import ExitStack

    with ExitStack() as ctx:
        x_sb = ctx.enter_context(nc.sbuf_tensor("x_sb", [128, DL * KC * B], fp8))
        w_sb = ctx.enter_context(nc.sbuf_tensor("w_sb", [128, DL * KC * O], fp8))
        thr_sb = ctx.enter_context(nc.sbuf_tensor("thr_sb", [B, DL * O], f16))
        out_sb = ctx.enter_context(nc.sbuf_tensor("out_sb", [B, DL * O], u8))
        warm_x = ctx.enter_context(nc.sbuf_tensor("warm_x", [128, B], fp8))
        warm_w = ctx.enter_context(nc.sbuf_tensor("warm_w", [128, O], fp8))
        psum = ctx.enter_context(nc.psum_tensor([B, NB * O], f32))
        sem_x = [ctx.enter_context(nc.semaphore(f"sem_x{k}")) for k in range(2)]
        sem_w = [ctx.enter_context(nc.semaphore(f"sem_w{k}")) for k in range(DL)]
        sem_thr = ctx.enter_context(nc.semaphore("sem_thr"))
        sem_pe = ctx.enter_context(nc.semaphore("sem_pe"))
        sem_dve = ctx.enter_context(nc.semaphore("sem_dve"))
        sem_out = ctx.enter_context(nc.semaphore("sem_out"))
        block = ctx.enter_context(nc.Block())
        xv = x_sb[:, :].rearrange("k (d c b) -> k d c b", d=DL, c=KC)
        wv = w_sb[:, :].rearrange("k (d c o) -> k d c o", d=DL, c=KC)

        DEPTH = 5  # w-chunk DMAs in flight (per-DMA BW ~100GB/s; need ~4-5)
        XH = DL * KC * B // 2

        @block.sync
        def _(sync):
            # One 256KB w chunk per direction.  Chained DEPTH-deep: enough
            # in-flight DMAs to hit aggregate HBM bandwidth while keeping
            # completions early and roughly in order.
            sync.dma_start(x_sb[:, 0:XH], xt_d[:, 0:XH]).then_inc(sem_x[0], 16)
            sync.dma_start(x_sb[:, XH:], xt_d[:, XH:]).then_inc(sem_x[1], 16)
            for p in range(DL):
                if p == 3:
                    sync.dma_start(thr_sb[:, :], thr_d[:, :]).then_inc(sem_thr, 16)
                if p >= DEPTH:
                    sync.wait_ge(sem_w[p - DEPTH], 16)
                sync.dma_start(
                    w_sb[:, p * KC * O : (p + 1) * KC * O], wt_d[p, :, :]
                ).then_inc(sem_w[p], 16)
            # Output in 4 slices so most of the store overlaps compute.
            for q in range(4):
                sync.wait_ge(sem_dve, (q + 1) * DL // 4)
                lo, hi = q * DL * O // 4, (q + 1) * DL * O // 4
                sync.dma_start(out_d[:, lo:hi], out_sb[:, lo:hi]).then_inc(
                    sem_out, 16
                )
            sync.wait_ge(sem_out, 64)

        @block.gpsimd
        def _(g):
            # cleanup: after everything retires, reset sems so the NEFF can
            # be re-executed
            g.wait_ge(sem_out, 64)
            all_sems = [*sem_x, *sem_w, sem_thr, sem_pe, sem_dve, sem_out]
            nums = sorted(s.num for s in all_sems)
            lo, hi = nums[0], nums[-1]
            assert nums == list(range(lo, hi + 1)), nums
            g.dma_reset(range(lo, hi + 1))
            g.sem_clear(range(lo, hi + 1))

        N_WARM = 10

        @block.tensor
        def _(t):
            # Warm the PE HAM clock gate on throwaway operands while the
            # first weight chunk streams in (~3.4us of busy time flips the
            # clock from 1.2 to 2.4 GHz).
            for _ in range(N_WARM):
                t.matmul(
                    psum[:, (NB - 1) * O : NB * O],
                    warm_x[:, :],
                    warm_w[:, :],
                    start=True,
                    stop=True,
                )
            t.wait_ge(sem_x[0], 16)
            for d in range(DL):
                if d == DL // 2:
                    t.wait_ge(sem_x[1], 16)
                t.wait_ge(sem_w[d], 16)
                if d >= NB:
                    t.wait_ge(sem_dve, d - NB + 1)
                bank = d % NB
                mm = None
                for c in range(KC):
                    mm = t.matmul(
                        psum[:, bank * O : (bank + 1) * O],
                        xv[:, d, c, :],
                        wv[:, d, c, :],
                        start=(c == 0),
                        stop=(c == KC - 1),
                    )
                mm.then_inc(sem_pe, 1)

        @block.vector
        def _(v):
            v.wait_ge(sem_thr, 16)
            for d in range(DL):
                v.wait_ge(sem_pe, d + 1)
                bank = d % NB
                v.tensor_tensor(
                    out=out_sb[:, d * O : (d + 1) * O],
                    in0=psum[:, bank * O : (bank + 1) * O],
                    in1=thr_sb[:, d * O : (d + 1) * O],
                    op=mybir.AluOpType.is_gt,
                ).then_inc(sem_dve, 1)

    nc.compile()
    return nc


def _get_nc():
    if "nc" not in _NC_CACHE:
        _NC_CACHE["nc"] = _build_bass()
    return _NC_CACHE["nc"]


def _prep_inputs(input, weight_noise, bias_noise):
    import ml_dtypes

    fp8 = ml_dtypes.float8_e4m3
    x = np.asarray(input).astype(np.int8)  # (B, D, I) in {0,1}
    w = np.asarray(weight_noise).astype(np.int8)  # (D, O, I)
    bias = np.asarray(bias_noise).astype(np.float32)  # (D, O)

    xs = (2 * x - 1).astype(fp8)  # +/-1
    ws = (2 * w - 1).astype(fp8)
    # dotpm is always an even integer, so comparing against the odd integer
    # 2*floor(thr/2)+1 gives identical results to comparing against thr —
    # and odd integers |t|<=513 are exact in fp16 (halves thr DMA traffic).
    thr = np.float32(2.0) * bias - np.float32(I)
    thr = (2.0 * np.floor(thr.astype(np.float64) / 2.0) + 1.0).astype(np.float16)

    in_maps = []
    for cidx in range(NCORES):
        dsl = slice(cidx * DL, (cidx + 1) * DL)
        # xt[k, d, c, b] = xs[b, d0+d, c*128+k]
        xt = xs[:, dsl, :].transpose(2, 1, 0)  # (I, DL, B)
        xt = xt.reshape(KC, 128, DL, B).transpose(1, 2, 0, 3)  # (k, d, c, b)
        xt = np.ascontiguousarray(xt).reshape(128, DL * KC * B)
        # wt[d, k, c, o] = ws[d0+d, o, c*128+k]
        wt = ws[dsl].transpose(0, 2, 1)  # (DL, I, O)
        wt = wt.reshape(DL, KC, 128, O).transpose(0, 2, 1, 3)  # (d, k, c, o)
        wt = np.ascontiguousarray(wt).reshape(DL, 128, KC * O)
        th = np.ascontiguousarray(
            np.broadcast_to(thr[dsl].reshape(1, DL * O), (B, DL * O))
        )
        in_maps.append({"xt": xt, "wt": wt, "thr": th})
    return in_maps


def _patch_walrus_args():
    """Cap the semaphore space walrus allocates: its NEFF epilogue clears
    every allocatable semaphore one instruction at a time (~6us for 256)."""
    from concourse import bass_utils as bu

    if getattr(bu, "_max_sem_patched", False):
        return
    orig = bu.get_walrus_args

    def patched(*a, **k):
        return ["--max-sem-num=64", *orig(*a, **k)]

    bu.get_walrus_args = patched
    bu._max_sem_patched = True


def kernel(input, weight_noise, bias_noise):
    from concourse import bass_utils

    _patch_walrus_args()
    in_maps = _prep_inputs(input, weight_noise, bias_noise)
    nc = _get_nc()
    res = bass_utils.run_bass_kernel_spmd(nc, in_maps, core_ids=list(range(NCORES)))
    outs = [np.asarray(r["out"]).reshape(B, DL, O) for r in res.results]
    full = np.concatenate(outs, axis=1)  # (B, D, O)
    return full.astype(bool)


# revision 30
# speedup vs baseline: 1.2826x; 1.0234x over previous
"""Binarized linear kernel for Trainium2 (8 NeuronCores).

Problem: per-direction binary "match count" GEMM.
  input        (B=64, D=128, I=512)  bool
  weight_noise (D=128, O=512, I=512) bool
  bias_noise   (D=128, O=512)        float32
  out[b,d,o] = (#matches(input[b,d,:], weight_noise[d,:,:]) > bias_noise[d,o])

Math: with +/-1 encoding x~=2x-1, w~=2w-1:
  matches = (I + sum_i x~ w~) / 2, so
  out = (dotpm > 2*bias - I), where dotpm is a single +/-1 GEMM per direction.
Host pre-encodes +/-1 in fp8_e4m3 (exact), precomputes thr = 2*bias - I (exact
in fp32: 2*bias is exact; subtracting 512 from values in [256,1024] is exact by
Sterbenz). PSUM accumulates fp32 exactly (integers <= 512), so the comparison
is bit-identical to the reference.

Sharding: D across the 8 cores (16 directions each), fully independent.

Implementation: raw bacc (no TileContext) — hand-rolled semaphore pipeline
to avoid Tile's ~15us of prologue/epilogue barriers and per-tile semaphore
churn.  Engine roles:
  Sync:   x DMA, 8x w-chunk DMAs (2 directions each), final out DMA
  GpSimd: thr row DMA + partition_broadcast to 64 partitions, sem cleanup
  PE:     per direction: 4 accumulating fp8 matmuls (K=4x128) into a PSUM bank
  DVE:    per direction: psum > thr compare -> uint8
"""

import numpy as np

import sys

for _p in ("/opt/trn_rl_repo",):
    if _p not in sys.path:
        sys.path.insert(0, _p)

B, D, O, I = 64, 128, 512, 512
NCORES = 8
DL = D // NCORES  # directions per core (16)
KC = I // 128     # contraction chunks of 128 (4)
NB = 8            # PSUM banks used (round robin over directions)

_NC_CACHE = {}


def _build_bass():
    import concourse.mybir as mybir
    from concourse import bacc
    import concourse.bass as bass

    fp8 = mybir.dt.float8e4
    f16 = mybir.dt.float16
    f32 = mybir.dt.float32
    u8 = mybir.dt.uint8

    nc = bacc.Bacc("TRN2")
    # DRAM layouts (host pre-arranged, all DMAs fully contiguous):
    #   xt [128, (d c b)]          : xt[k, d, c, b] = xs[b, d0+d, c*128+k]
    #   wt [8, 128, (j c o)]       : wt[p, k, j, c, o] = ws[d0+2p+j, o, c*128+k]
    #   thr [1, (d o)]             : 2*bias - I
    #   out [64, (d o)]
    xt_d = nc.dram_tensor("xt", [128, DL * KC * B], fp8, kind="ExternalInput")
    wt_d = nc.dram_tensor("wt", [DL, 128, KC * O], fp8, kind="ExternalInput")
    thr_d = nc.dram_tensor("thr", [B, DL * O], f16, kind="ExternalInput")
    out_d = nc.dram_tensor("out", [B, DL * O], u8, kind="ExternalOutput")

    from contextlib import ExitStack

    with ExitStack() as ctx:
        x_sb = ctx.enter_context(nc.sbuf_tensor("x_sb", [128, DL * KC * B], fp8))
        w_sb = ctx.enter_context(nc.sbuf_tensor("w_sb", [128, DL * KC * O], fp8))
        thr_sb = ctx.enter_context(nc.sbuf_tensor("thr_sb", [B, DL * O], f16))
        out_sb = ctx.enter_context(nc.sbuf_tensor("out_sb", [B, DL * O], u8))
        warm_x = ctx.enter_context(nc.sbuf_tensor("warm_x", [128, B], fp8))
        warm_w = ctx.enter_context(nc.sbuf_tensor("warm_w", [128, O], fp8))
        psum = ctx.enter_context(nc.psum_tensor([B, NB * O], f32))
        sem_x = [ctx.enter_context(nc.semaphore(f"sem_x{k}")) for k in range(2)]
        sem_w = [ctx.enter_context(nc.semaphore(f"sem_w{k}")) for k in range(DL)]
        sem_thr = ctx.enter_context(nc.semaphore("sem_thr"))
        sem_pe = ctx.enter_context(nc.semaphore("sem_pe"))
        sem_dve = ctx.enter_context(nc.semaphore("sem_dve"))
        sem_out = ctx.enter_context(nc.semaphore("sem_out"))
        block = ctx.enter_context(nc.Block())
        xv = x_sb[:, :].rearrange("k (d c b) -> k d c b", d=DL, c=KC)
        wv = w_sb[:, :].rearrange("k (d c o) -> k d c o", d=DL, c=KC)

        DEPTH = 8  # w-chunk DMAs in flight: all 8 HWDGE queues stay fed
        XH = DL * KC * B // 2

        @block.sync
        def _(sync):
            # One 256KB w chunk per direction.  First 8 chunks issued
            # immediately (fills every HWDGE queue -> max aggregate BW);
            # later chunks chained DEPTH-deep.
            sync.dma_start(x_sb[:, 0:XH], xt_d[:, 0:XH]).then_inc(sem_x[0], 16)
            sync.dma_start(x_sb[:, XH:], xt_d[:, XH:]).then_inc(sem_x[1], 16)
            for p in range(DL):
                if p == 3:
                    sync.dma_start(thr_sb[:, :], thr_d[:, :]).then_inc(sem_thr, 16)
                if p >= DEPTH:
                    sync.wait_ge(sem_w[p - DEPTH], 16)
                sync.dma_start(
                    w_sb[:, p * KC * O : (p + 1) * KC * O], wt_d[p, :, :]
                ).then_inc(sem_w[p], 16)
            # Output in 4 slices so most of the store overlaps compute.
            for q in range(4):
                sync.wait_ge(sem_dve, (q + 1) * DL // 4)
                lo, hi = q * DL * O // 4, (q + 1) * DL * O // 4
                sync.dma_start(out_d[:, lo:hi], out_sb[:, lo:hi]).then_inc(
                    sem_out, 16
                )
            sync.wait_ge(sem_out, 64)

        @block.gpsimd
        def _(g):
            # cleanup: after everything retires, reset sems so the NEFF can
            # be re-executed
            g.wait_ge(sem_out, 64)
            all_sems = [*sem_x, *sem_w, sem_thr, sem_pe, sem_dve, sem_out]
            nums = sorted(s.num for s in all_sems)
            lo, hi = nums[0], nums[-1]
            assert nums == list(range(lo, hi + 1)), nums
            g.dma_reset(range(lo, hi + 1))
            g.sem_clear(range(lo, hi + 1))

        N_WARM = 14

        @block.tensor
        def _(t):
            # Warm the PE HAM clock gate on throwaway operands while the
            # first weight chunk streams in (~3.4us of busy time flips the
            # clock from 1.2 to 2.4 GHz).
            for _ in range(N_WARM):
                t.matmul(
                    psum[:, (NB - 1) * O : NB * O],
                    warm_x[:, :],
                    warm_w[:, :],
                    start=True,
                    stop=True,
                )
            t.wait_ge(sem_x[0], 16)
            for d in range(DL):
                if d == DL // 2:
                    t.wait_ge(sem_x[1], 16)
                t.wait_ge(sem_w[d], 16)
                if d >= NB:
                    t.wait_ge(sem_dve, d - NB + 1)
                bank = d % NB
                mm = None
                for c in range(KC):
                    mm = t.matmul(
                        psum[:, bank * O : (bank + 1) * O],
                        xv[:, d, c, :],
                        wv[:, d, c, :],
                        start=(c == 0),
                        stop=(c == KC - 1),
                    )
                mm.then_inc(sem_pe, 1)

        @block.vector
        def _(v):
            v.wait_ge(sem_thr, 16)
            for d in range(DL):
                v.wait_ge(sem_pe, d + 1)
                bank = d % NB
                v.tensor_tensor(
                    out=out_sb[:, d * O : (d + 1) * O],
                    in0=psum[:, bank * O : (bank + 1) * O],
                    in1=thr_sb[:, d * O : (d + 1) * O],
                    op=mybir.AluOpType.is_gt,
                ).then_inc(sem_dve, 1)

    nc.compile()
    return nc


def _get_nc():
    if "nc" not in _NC_CACHE:
        _NC_CACHE["nc"] = _build_bass()
    return _NC_CACHE["nc"]


def _prep_inputs(input, weight_noise, bias_noise):
    import ml_dtypes

    fp8 = ml_dtypes.float8_e4m3
    x = np.asarray(input).astype(np.int8)  # (B, D, I) in {0,1}
    w = np.asarray(weight_noise).astype(np.int8)  # (D, O, I)
    bias = np.asarray(bias_noise).astype(np.float32)  # (D, O)

    xs = (2 * x - 1).astype(fp8)  # +/-1
    ws = (2 * w - 1).astype(fp8)
    # dotpm is always an even integer, so comparing against the odd integer
    # 2*floor(thr/2)+1 gives identical results to comparing against thr —
    # and odd integers |t|<=513 are exact in fp16 (halves thr DMA traffic).
    thr = np.float32(2.0) * bias - np.float32(I)
    thr = (2.0 * np.floor(thr.astype(np.float64) / 2.0) + 1.0).astype(np.float16)

    in_maps = []
    for cidx in range(NCORES):
        dsl = slice(cidx * DL, (cidx + 1) * DL)
        # xt[k, d, c, b] = xs[b, d0+d, c*128+k]
        xt = xs[:, dsl, :].transpose(2, 1, 0)  # (I, DL, B)
        xt = xt.reshape(KC, 128, DL, B).transpose(1, 2, 0, 3)  # (k, d, c, b)
        xt = np.ascontiguousarray(xt).reshape(128, DL * KC * B)
        # wt[d, k, c, o] = ws[d0+d, o, c*128+k]
        wt = ws[dsl].transpose(0, 2, 1)  # (DL, I, O)
        wt = wt.reshape(DL, KC, 128, O).transpose(0, 2, 1, 3)  # (d, k, c, o)
        wt = np.ascontiguousarray(wt).reshape(DL, 128, KC * O)
        th = np.ascontiguousarray(
            np.broadcast_to(thr[dsl].reshape(1, DL * O), (B, DL * O))
        )
        in_maps.append({"xt": xt, "wt": wt, "thr": th})
    return in_maps


def _patch_walrus_args():
    """Cap the semaphore space walrus allocates: its NEFF epilogue clears
    every allocatable semaphore one instruction at a time (~6us for 256)."""
    from concourse import bass_utils as bu

    if getattr(bu, "_max_sem_patched", False):
        return
    orig = bu.get_walrus_args

    def patched(*a, **k):
        return ["--max-sem-num=64", *orig(*a, **k)]

    bu.get_walrus_args = patched
    bu._max_sem_patched = True


def kernel(input, weight_noise, bias_noise):
    from concourse import bass_utils

    _patch_walrus_args()
    in_maps = _prep_inputs(input, weight_noise, bias_noise)
    nc = _get_nc()
    res = bass_utils.run_bass_kernel_spmd(nc, in_maps, core_ids=list(range(NCORES)))
    outs = [np.asarray(r["out"]).reshape(B, DL, O) for r in res.results]
    full = np.concatenate(outs, axis=1)  # (B, D, O)
    return full.astype(bool)
